# revision 37
# baseline (speedup 1.0000x reference)
"""Parametric Bass/Tile attention-layer kernel for TRN2 (8-core data parallel).

Per-core computation (BP batch elements each):
  h      = (x @ w_in.T + b_in + te) * scale          [T, E]
  scores = h @ keys + (-1e9 * mask)                  [T, S]
  attn   = softmax(scores, axis=-1)                  -> output
  ctx    = (attn @ values) * sqrt(valid)             [T, E]
  out    = (ctx @ w_out.T + b_out + x) * scale       -> output

Layout strategy: every tensor the PE needs with the contraction dim on
partitions is pre-transposed on the HOST (numpy) and streamed in natively:
x^T and te^T per batch, (w_in*scale)^T and w_out^T once. The device does
zero layout work for the h-path; the only on-chip transpose is attn^T
(bf16, 1 cyc/row on the PE).

Precision: h/scores path f32r end-to-end (softmax input errors amplify by
sqrt(E) through the exp); downstream of the softmax runs bf16 (attn values
are in [0,1]; relative error stays relative through attn@V and the
out-projection, and the residual dominates the output scale).

Host pre-folds: w_in *= scale, te_eff = (te + b_in) * scale, w_out/values
cast to bf16, residual read from bf16 x.

PE work per 256-row block: 4 matmuls at 16384 cyc each + attn^T (2048)
= 67.6k cyc = 28.2us; 8 blocks/core ~= 225us PE-busy (HAM-warm floor).
The te and residual additions ride DVE tensor_tensor ops fused into the
PSUM->SBUF drains (zero extra passes). Software pipelining: mm1 of block
n+1 is emitted before the attn^T/mm3/mm4 tail of block n so the PE covers
the softmax ACT/DVE latency chain; x^T/te^T prefetch two blocks ahead.
DMA queues: always-ready loads on the two HWDGE rings (xT/teT/keys/weights,
balanced), data-dependent stores and bulk residual/values loads on SWDGE —
HWDGE rings are FIFO and a waiting transfer blocks everything behind it.
"""

import math
import os
import sys
import tempfile

os.environ.setdefault("NEURON_COMPILE_CACHE_URL",
                      tempfile.mkdtemp(prefix="neuroncache_"))

sys.path.insert(0, "/opt/trn_rl_repo")
sys.path.insert(0, "/opt/trn_rl_repo/concourse")

from contextlib import ExitStack, nullcontext

import concourse.bass as bass
import concourse.tile as tile
from concourse import bacc, mybir

P = 128
f32 = mybir.dt.float32
f32r = mybir.dt.float32r
bf16 = mybir.dt.bfloat16
u8 = mybir.dt.uint8
AF = mybir.ActivationFunctionType
ALU = mybir.AluOpType

NEG_BIG = -1.0e9


def build_attn(n_cores=8, BP=2, T=1024, S=1024, C=1024, E=1024, TBLK=256,
               scale=math.sqrt(0.5), psum_bufs=8, loop_n=None,
               internal_io=False):
    CT, ET, ST = C // P, E // P, S // P
    NB = T // TBLK
    TPB = TBLK // P
    SN = min(512, S)
    CN = min(512, C)

    nc = bacc.Bacc("TRN2", target_bir_lowering=False, debug=False,
                   num_devices=n_cores)

    ki = "Internal" if internal_io else "ExternalInput"
    ko = "Internal" if internal_io else "ExternalOutput"
    xT_d = nc.dram_tensor("xT", [BP, C, T], f32r, kind=ki).ap()
    x_d = nc.dram_tensor("x", [BP, T, C], f32r, kind=ki).ap()
    teT_d = nc.dram_tensor("teT", [BP, E, T], f32, kind=ki).ap()
    k_d = nc.dram_tensor("keys", [BP, E, S], f32r, kind=ki).ap()
    v_d = nc.dram_tensor("values16", [BP, S, E], bf16, kind=ki).ap()
    m_d = nc.dram_tensor("mask", [BP, S], u8, kind=ki).ap()
    wiT_dram = nc.dram_tensor("wiT_h", [C, E], f32r, kind=ki).ap()
    woT_dram = nc.dram_tensor("woT_h", [E, C], bf16, kind=ki).ap()
    ssv_d = nc.dram_tensor("ssv", [BP, 1], f32, kind=ki).ap()
    out_d = nc.dram_tensor("out", [BP, T, C], f32, kind=ko).ap()
    attn_d = nc.dram_tensor("attn", [BP, T, S], bf16, kind=ko).ap()
    tick_d = (nc.dram_tensor("tick", [1, 8], f32, kind="ExternalOutput").ap()
              if internal_io else None)

    with tile.TileContext(nc) as tc, ExitStack() as ctx:
        consts = ctx.enter_context(tc.tile_pool(name="consts", bufs=1))
        batchp = ctx.enter_context(tc.tile_pool(name="batchp", bufs=1))
        blocks = ctx.enter_context(tc.tile_pool(name="blocks", bufs=1))
        tiles = ctx.enter_context(tc.tile_pool(name="tiles", bufs=2))
        stats = ctx.enter_context(tc.tile_pool(name="stats", bufs=2))
        psum = ctx.enter_context(
            tc.tile_pool(name="psum", bufs=psum_bufs, space="PSUM"))

        if internal_io:
            # zero every input region once (outside the timing loop) so the
            # looped kernel computes on finite data instead of DRAM garbage
            zt = consts.tile([P, 2048], f32, tag="zt")
            nc.gpsimd.memset(zt, 0.0)
            CH = P * 2048
            for ten, nelem in ((xT_d, BP * C * T), (x_d, BP * T * C),
                               (teT_d, BP * E * T), (k_d, BP * E * S),
                               (wiT_dram, C * E)):
                for off in range(0, nelem, CH):
                    dst = bass.AP(tensor=ten.tensor, offset=ten.offset + off,
                                  ap=[[2048, P], [1, 2048]])
                    nc.gpsimd.dma_start(dst, zt[:])
            for ten, nelem in ((v_d, BP * S * E), (woT_dram, E * C)):
                for off in range(0, nelem, CH):
                    dst = bass.AP(tensor=ten.tensor, offset=ten.offset + off,
                                  ap=[[2048, P], [1, 2048]])
                    nc.gpsimd.dma_start(dst, zt[:])
            mdst = bass.AP(tensor=m_d.tensor, offset=m_d.offset,
                           ap=[[S, BP], [1, S]])
            nc.gpsimd.dma_start(mdst, zt[0:BP, 0:S])
            sdst = bass.AP(tensor=ssv_d.tensor, offset=ssv_d.offset,
                           ap=[[1, BP], [1, 1]])
            nc.gpsimd.dma_start(sdst, zt[0:BP, 0:1])

        loop_cm = tc.For_i(0, loop_n) if loop_n else nullcontext()
        ctx.enter_context(loop_cm)

        _ps_ctr = [0]

        def ps_tile(w, dt=f32):
            _ps_ctr[0] += 1
            return psum.tile([P, w], dt, tag="ps", bufs=psum_bufs,
                             name=f"ps{_ps_ctr[0]}")

        # ---- constants ----
        ident = consts.tile([P, P], f32, tag="ident")
        nc.gpsimd.memset(ident, 0.0)
        nc.gpsimd.affine_select(out=ident, in_=ident,
                                compare_op=ALU.not_equal, fill=1.0,
                                base=0, pattern=[[-1, P]], channel_multiplier=1)
        ident_b = consts.tile([P, P], bf16, tag="ident_b")
        nc.vector.tensor_copy(ident_b[:], ident[:])

        # ---- weights: host-pretransposed, direct loads ----
        wiT = [consts.tile([P, E], f32r, tag=f"wiT{i}", name=f"wiT{i}")
               for i in range(CT)]
        woT = [consts.tile([P, C], bf16, tag=f"woT{i}", name=f"woT{i}")
               for i in range(ET)]

        def prep_wi():
            WCH = 256
            for ech in range(E // WCH):
                for ct in range(CT):
                    eng = nc.sync if ct % 2 == 0 else nc.scalar
                    eng.dma_start(
                        wiT[ct][:, ech * WCH:(ech + 1) * WCH],
                        wiT_dram[ct * P:(ct + 1) * P,
                                 ech * WCH:(ech + 1) * WCH])

        def prep_wo():
            for et in range(ET):
                eng = nc.sync if et % 2 == 0 else nc.scalar
                eng.dma_start(woT[et][:], woT_dram[et * P:(et + 1) * P, :])

        # ---- per-batch state ----
        state = {}

        def batch_prep_a(b):
            m8 = batchp.tile([1, S], u8, tag="m8")
            nc.sync.dma_start(m8[:], m_d[b:b + 1, :])
            keys_r = [batchp.tile([P, S], f32r, tag=f"keys{i}",
                                  name=f"keys{i}") for i in range(ET)]
            for sch in range(S // SN):
                for et in range(ET):
                    eng = nc.sync if et % 2 == 0 else nc.scalar
                    eng.dma_start(keys_r[et][:, sch * SN:(sch + 1) * SN],
                                  k_d[b, et * P:(et + 1) * P,
                                      sch * SN:(sch + 1) * SN])
            state[b] = [keys_r, None, None, None]

            maskrep = batchp.tile([P, S], f32, tag="maskrep")
            nc.vector.tensor_scalar_mul(maskrep[0:1, :], m8[:], NEG_BIG)
            nc.gpsimd.partition_broadcast(maskrep[:], maskrep[0:1, :])
            nvalid = batchp.tile([1, 1], f32, tag="nvalid")
            nc.sync.dma_start(nvalid[:], ssv_d[b:b + 1, :])
            ssv_rep = batchp.tile([P, 1], f32, tag="ssv_rep")
            nc.gpsimd.partition_broadcast(ssv_rep[:], nvalid[:])
            state[b][2] = maskrep
            state[b][3] = ssv_rep

        def batch_prep_b(b):
            vals_r = [batchp.tile([P, E], bf16, tag=f"vals{i}",
                                  name=f"vals{i}") for i in range(ST)]
            for st in range(ST):
                nc.gpsimd.dma_start(vals_r[st][:],
                                    v_d[b, st * P:(st + 1) * P, :])
            state[b][1] = vals_r

        def stage_x(b, blk):
            """Residual tiles (bf16 native) + x^T block (f32r, one DMA)."""
            t0 = blk * TBLK
            xT = blocks.tile([P, CT, TBLK], f32r, tag="xT", bufs=2)
            src = bass.AP(
                tensor=xT_d.tensor,
                offset=xT_d.offset + (b * C + 0) * T + t0,
                ap=[[T, P], [P * T, CT], [1, TBLK]])
            nc.sync.dma_start(xT[:], src)
            x_t = [tiles.tile([P, C], f32r, tag="x", bufs=2 * TPB,
                              name=f"x{i}") for i in range(TPB)]
            for tt in range(TPB):
                nc.gpsimd.dma_start(
                    x_t[tt][:], x_d[b, t0 + tt * P:t0 + (tt + 1) * P, :])
            return x_t, xT

        def stage_te(b, blk):
            """te^T block (f32, one DMA)."""
            t0 = blk * TBLK
            teT = blocks.tile([P, ET, TBLK], f32, tag="teT", bufs=2)
            src = bass.AP(
                tensor=teT_d.tensor,
                offset=teT_d.offset + (b * E + 0) * T + t0,
                ap=[[T, P], [P * T, ET], [1, TBLK]])
            nc.scalar.dma_start(teT[:], src)
            return teT

        def stage_mm1(b, blk, xT, teT):
            hT = blocks.tile([P, ET, TBLK], f32r, tag="hT", bufs=2)
            for et in range(ET):
                ps = ps_tile(TBLK)
                for ct in range(CT):
                    nc.tensor.matmul(ps[:], wiT[ct][:, et * P:(et + 1) * P],
                                     xT[:, ct, :], start=(ct == 0),
                                     stop=(ct == CT - 1),
                                     skip_group_check=True)
                nc.vector.tensor_tensor(out=hT[:, et, :], in0=ps[:],
                                        in1=teT[:, et, :], op=ALU.add)
            return hT

        def stage_mm2(b, blk, hT):
            keys_r, _, maskrep, _ = state[b]
            t0 = blk * TBLK
            sc16_t = []
            for tt in range(TPB):
                sc = tiles.tile([P, S], f32, tag="sc", bufs=TPB, name="sc")
                sc16 = tiles.tile([P, S], bf16, tag="sc16", bufs=TPB,
                                  name="sc16")
                sc16_t.append(sc16)
                nmx = stats.tile([P, S // SN], f32, tag="nmx", bufs=4)
                pss = [ps_tile(SN) for _ in range(S // SN)]
                for et in range(ET):
                    for sch in range(S // SN):
                        nc.tensor.matmul(
                            pss[sch][:], hT[:, et, tt * P:(tt + 1) * P],
                            keys_r[et][:, sch * SN:(sch + 1) * SN],
                            start=(et == 0), stop=(et == ET - 1),
                            skip_group_check=True)
                for sch in range(S // SN):
                    # masked copy out of psum on DVE; raw negmax from
                    # psum in parallel (raw max >= masked max, which is all
                    # softmax stability needs)
                    nc.vector.tensor_tensor(
                        out=sc[:, sch * SN:(sch + 1) * SN], in0=pss[sch][:],
                        in1=maskrep[:, sch * SN:(sch + 1) * SN], op=ALU.add)
                    nc.vector.tensor_reduce(nmx[:, sch:sch + 1], pss[sch][:],
                                            axis=mybir.AxisListType.X,
                                            op=ALU.max, negate=True)
                negmax = stats.tile([P, 1], f32, tag="negmax")
                nc.vector.tensor_reduce(negmax[:], nmx[:],
                                        axis=mybir.AxisListType.X,
                                        op=ALU.min)
                sumexp = stats.tile([P, 1], f32, tag="sumexp")
                nc.scalar.activation(sc[:], sc[:], AF.Exp,
                                     bias=negmax[:, 0:1], scale=1.0,
                                     accum_out=sumexp[:, 0:1])
                recip = stats.tile([P, 1], f32, tag="recip")
                nc.vector.reciprocal(recip[:], sumexp[:])
                nc.vector.tensor_scalar_mul(sc16[:], sc[:], recip[:, 0:1])
                nc.gpsimd.dma_start(
                    attn_d[b, t0 + tt * P:t0 + (tt + 1) * P, :], sc16[:])
            return sc16_t

        def stage_tail(b, blk, sc16_t, x_t, final=False):
            _, vals_r, _, ssv_rep = state[b]
            t0 = blk * TBLK
            aT = blocks.tile([P, ST, TBLK], bf16, tag="aT")
            for st in range(ST):
                ps = ps_tile(TBLK, bf16)
                for tt in range(TPB):
                    nc.tensor.matmul(ps[:, tt * P:(tt + 1) * P],
                                     sc16_t[tt][:, st * P:(st + 1) * P],
                                     ident_b[:], is_transpose=True,
                                     start=(tt == 0), stop=(tt == TPB - 1),
                                     skip_group_check=True)
                nc.scalar.copy(aT[:, st, :], ps[:])

            cxT = blocks.tile([P, ET, TBLK], bf16, tag="cxT")
            for et in range(ET):
                ps = ps_tile(TBLK)
                for st in range(ST):
                    nc.tensor.matmul(ps[:], vals_r[st][:, et * P:(et + 1) * P],
                                     aT[:, st, :], start=(st == 0),
                                     stop=(st == ST - 1))
                nc.scalar.activation(cxT[:, et, :], ps[:], AF.Copy,
                                     scale=ssv_rep[:, 0:1])

            for tt in range(TPB):
                ot = tiles.tile([P, C], f32, tag="ot", bufs=1, name="ot")
                pss = [ps_tile(CN) for _ in range(C // CN)]
                for et in range(ET):
                    for cch in range(C // CN):
                        nc.tensor.matmul(
                            pss[cch][:], cxT[:, et, tt * P:(tt + 1) * P],
                            woT[et][:, cch * CN:(cch + 1) * CN],
                            start=(et == 0), stop=(et == ET - 1),
                            skip_group_check=True)
                for cch in range(C // CN):
                    # residual folded in on the psum->sbuf copy (x_t holds
                    # scale*(x + b_out/scale) from the host)
                    nc.vector.tensor_tensor(
                        out=ot[:, cch * CN:(cch + 1) * CN], in0=pss[cch][:],
                        in1=x_t[tt][:, cch * CN:(cch + 1) * CN], op=ALU.add)
                    if final:
                        eng = (nc.gpsimd, nc.sync, nc.scalar)[(tt * 2 + cch) % 3]
                        eng.dma_start(
                            out_d[b, t0 + tt * P:t0 + (tt + 1) * P,
                                  cch * CN:(cch + 1) * CN],
                            ot[:, cch * CN:(cch + 1) * CN])
                if not final:
                    nc.gpsimd.dma_start(
                        out_d[b, t0 + tt * P:t0 + (tt + 1) * P, :], ot[:])

        # ---- pipelined emission over (batch, block) ----
        # Startup DMAs land in consumption order (x0/te0, w_in, keys, vals,
        # w_out). Steady state: mm1 of block n+1 between mm2(n) and tail(n)
        # so the PE covers the softmax latency; x/te loads prefetch 2 blocks
        # ahead.
        seq = [(b, blk) for b in range(BP) for blk in range(NB)]
        n = len(seq)
        sx = {0: stage_x(*seq[0])}
        ste = {0: stage_te(*seq[0])}
        prep_wi()
        hT = {0: stage_mm1(*seq[0], sx[0][1], ste[0])}
        batch_prep_a(0)
        sc = {0: stage_mm2(*seq[0], hT[0])}
        sx[1] = stage_x(*seq[1])
        ste[1] = stage_te(*seq[1])
        batch_prep_b(0)
        prep_wo()
        hT[1] = stage_mm1(*seq[1], sx[1][1], ste[1])
        stage_tail(*seq[0], sc[0], sx[0][0])
        sx[2] = stage_x(*seq[2])
        ste[2] = stage_te(*seq[2])
        for i, (b, blk) in enumerate(seq):
            if i == 0:
                continue
            if i not in sc:
                sc[i] = stage_mm2(b, blk, hT[i])
            if i + 2 < n:
                sx[i + 2] = stage_x(*seq[i + 2])
                ste[i + 2] = stage_te(*seq[i + 2])
            if i + 1 < n:
                nb, nblk = seq[i + 1]
                if nb != b:
                    batch_prep_a(nb)
                    batch_prep_b(nb)
                hT[i + 1] = stage_mm1(*seq[i + 1], sx[i + 1][1], ste[i + 1])
                if i + 1 == n - 1:
                    # last block: emit its mm2 now so tail(i) hides the
                    # softmax latency that no later mm1 can cover
                    sc[i + 1] = stage_mm2(*seq[i + 1], hT[i + 1])
            stage_tail(b, blk, sc[i], sx[i][0], final=(i == n - 1))

        if tick_d is not None:
            tick = stats.tile([1, 8], f32, tag="tick")
            nc.vector.tensor_scalar_mul(tick[:], ident[0:1, 0:8], 1.0)
            nc.sync.dma_start(tick_d, tick[:])

    nc.compile()
    return nc


N_CORES = 8
B, T, S, C, E = 16, 1024, 1024, 1024, 1024
BP = B // N_CORES

_NC = None
_RUNNER = None


def _make_runner(nc):
    """Reusable jitted 8-core runner (modeled on
    concourse.bass2jax.run_bass_via_pjrt, cached across calls)."""
    import jax
    import numpy as np
    from jax.sharding import Mesh, PartitionSpec
    from jax.experimental.shard_map import shard_map
    from concourse.bass2jax import (_bass_exec_p, install_neuronx_cc_hook,
                                    partition_id_tensor)

    install_neuronx_cc_hook()
    partition_name = nc.partition_id_tensor.name if nc.partition_id_tensor else None

    in_names, out_names, out_avals, zero_shapes = [], [], [], []
    for alloc in nc.m.functions[0].allocations:
        if not isinstance(alloc, mybir.MemoryLocationSet):
            continue
        name = alloc.memorylocations[0].name
        if alloc.kind == "ExternalInput":
            if name != partition_name:
                in_names.append(name)
        elif alloc.kind == "ExternalOutput":
            shape = tuple(alloc.tensor_shape)
            dtype = mybir.dt.np(alloc.dtype)
            out_names.append(name)
            out_avals.append(jax.core.ShapedArray(shape, dtype))
            zero_shapes.append((shape, dtype))
    n_params = len(in_names)
    all_in_names = list(in_names) + list(out_names)
    if partition_name is not None:
        all_in_names.append(partition_name)

    def _body(*args):
        operands = list(args)
        if partition_name is not None:
            operands.append(partition_id_tensor())
        outs = _bass_exec_p.bind(
            *operands, out_avals=tuple(out_avals), in_names=tuple(all_in_names),
            out_names=tuple(out_names), lowering_input_output_aliases=(),
            sim_require_finite=True, sim_require_nnan=True, nc=nc)
        return tuple(outs)

    devices = jax.devices()[:N_CORES]
    mesh = Mesh(np.asarray(devices), ("core",))
    n_outs = len(out_names)
    sharded = jax.jit(
        shard_map(_body, mesh=mesh,
                  in_specs=(PartitionSpec("core"),) * (n_params + n_outs),
                  out_specs=(PartitionSpec("core"),) * n_outs,
                  check_rep=False),
        keep_unused=True)
    zeros = [np.zeros((N_CORES * s[0], *s[1:]), d) for s, d in zero_shapes]

    def run(in_maps):
        concat_in = [
            np.concatenate([np.asarray(m[name]) for m in in_maps], axis=0)
            for name in in_names
        ]
        out_arrs = sharded(*concat_in, *zeros)
        jax.block_until_ready(out_arrs)
        return {name: np.asarray(out_arrs[i]) for i, name in enumerate(out_names)}

    return run


def kernel(x, target_embedding, enc_keys, enc_values, encoder_padding_mask,
           w_in, b_in, w_out, b_out):
    import numpy as np
    import ml_dtypes
    global _NC, _RUNNER
    if _NC is None:
        _NC = build_attn(n_cores=N_CORES, BP=BP, T=T, S=S, C=C, E=E, TBLK=256)
        _RUNNER = _make_runner(_NC)

    scale = np.float32(math.sqrt(0.5))
    b16 = ml_dtypes.bfloat16

    x = np.asarray(x, dtype=np.float32)
    xT = np.ascontiguousarray(x.transpose(0, 2, 1))
    xr = np.ascontiguousarray(
        (x + np.asarray(b_out, dtype=np.float32)[None, None, :]) * scale)
    te = np.asarray(target_embedding, dtype=np.float32)
    te_eff = (te + np.asarray(b_in, dtype=np.float32)[None, None, :]) * scale
    teT = np.ascontiguousarray(te_eff.transpose(0, 2, 1))
    keys = np.ascontiguousarray(np.asarray(enc_keys, dtype=np.float32))
    values16 = np.ascontiguousarray(
        np.asarray(enc_values, dtype=np.float32).astype(b16))
    mask = np.ascontiguousarray(np.asarray(encoder_padding_mask)).astype(np.uint8)
    wiT_h = np.ascontiguousarray(
        (np.asarray(w_in, dtype=np.float32) * scale).T)
    woT_h = np.ascontiguousarray(
        np.asarray(w_out, dtype=np.float32).astype(b16).T)
    b_out = np.ascontiguousarray(np.asarray(b_out, dtype=np.float32)).reshape(1, C)
    valid = (S - mask.astype(np.int64).sum(axis=1)).astype(np.float32)
    ssv = np.ascontiguousarray((np.sqrt(valid) * scale).reshape(B, 1))

    in_maps = []
    for c in range(N_CORES):
        sl = slice(c * BP, (c + 1) * BP)
        in_maps.append({
            "xT": xT[sl], "x": xr[sl], "teT": teT[sl],
            "keys": keys[sl], "values16": values16[sl],
            "mask": mask[sl], "wiT_h": wiT_h, "woT_h": woT_h,
            "b_out": b_out, "ssv": ssv[sl],
        })

    res = _RUNNER(in_maps)
    out = res["out"].reshape(B, T, C).astype(np.float32)
    attn = res["attn"].reshape(B, T, S).astype(np.float32)
    return out, attn
